# revision 7
# baseline (speedup 1.0000x reference)
"""Multi-head attention (B=2, S=4096, D=1024, H=16, HD=64) on 8 trn2 cores.

Sharding: core c -> batch b = c//4, head-group g = c%4 (4 heads per core).
Each core: Q/K/V projections for its heads on its batch, attention, and the
partial output ctx @ Wo[rows of its heads]. Host sums the 4 partials per
batch and adds bo.

Per-core kernel layout choices:
  - X^T [D, tok] built on-chip via PE transposes (DMA transpose is 2B-only).
  - Q^T, K^T stored [dq, tok] with each head-pair packed on partitions
    (head 2p   -> partitions  0:64,
     head 2p+1 -> partitions 64:128) so scores matmuls run as two
    concurrent 64-row PE tiles (full PE rate despite HD=64 contraction).
  - Scores computed transposed: S^T[k, q] = K^T.T @ Q^T. Softmax skips the
    max-subtraction (scores are O(5), exp cannot overflow) so exp is a
    single scalar-engine pass PSUM->SBUF with the 1/8 scale folded in.
  - V stored token-major with a ones column appended per head (V'[k, 65]).
    PV matmul (row-tiled over k halves) then yields ctx^T[d, q] and the
    softmax denominator row in one accumulation.
  - Normalization: transpose ctx''[65, q-tile] -> [q, 65], multiply by
    reciprocal of the sums column (per-partition scalar), transpose back,
    feed as lhsT of the Wo matmul.
All matmuls run as float32r (full-rate fp32 at free dim >= 256).
"""

import os
from contextlib import ExitStack

import numpy as np

os.environ.setdefault("MYCRO_LOCAL_CACHE", "1")

import concourse.bass as bass
import concourse.tile as tile
from concourse import bacc, mybir
from concourse.bass_utils import run_bass_kernel_spmd
from concourse.masks import make_identity

F32 = mybir.dt.float32
F32R = mybir.dt.float32r
AF = mybir.ActivationFunctionType

S = 4096          # sequence length
D = 1024          # model dim
HC = 4            # heads per core
HD = 64           # head dim
DC = HC * HD      # 256 per-core projection width
NP = HC // 2      # head pairs per core
KT = S // 128     # 32 k-tiles
QC = S // 512     # 8 q-chunks of 512
TC = S // 512     # 8 tok-chunks of 512
SCALE = 1.0 / 8.0


def r(ap):
    return ap.bitcast(F32R)


def _emit(ctx: ExitStack, tc: tile.TileContext, ins: dict, out: bass.AP):
    nc = tc.nc
    X, Wq, bq, Wk, bk, Wv, bv, Wo = (
        ins["X"], ins["Wq"], ins["bq"], ins["Wk"], ins["bk"], ins["Wv"],
        ins["bv"], ins["Wo"],
    )

    const = ctx.enter_context(tc.tile_pool(name="const", bufs=1))
    ident = const.tile([128, 128], F32)
    make_identity(nc, ident[:])

    # Weights DMA'd as fp32 then converted to f32r (fp32r matmul operands
    # must be produced pre-rounded by a compute engine; DMA cannot round).
    wq_sb = const.tile([128, 8 * DC], F32R, tag="wq")
    wk_sb = const.tile([128, 8 * DC], F32R, tag="wk")
    wv_sb = const.tile([128, 8 * DC], F32R, tag="wv")
    wo_sb = const.tile([128, 2 * D], F32R, tag="wo")
    with tc.tile_pool(name="wstage", bufs=2) as wst:
        for dst, src, nchunks in ((wq_sb, Wq, 8), (wk_sb, Wk, 8),
                                  (wv_sb, Wv, 8), (wo_sb, Wo, 2)):
            stg = wst.tile([128, dst.shape[1]], F32, tag="wstg",
                           name=f"wstg_{src.name}")
            nc.sync.dma_start(stg[:].rearrange("p (c d) -> p c d", c=nchunks),
                              src.rearrange("(c p) d -> p c d", p=128))
            nc.vector.tensor_copy(dst[:], stg[:])
    bq_sb = const.tile([128, 2], F32, tag="bq")
    bk_sb = const.tile([128, 2], F32, tag="bk")
    nc.sync.dma_start(bq_sb[:], bq.rearrange("(c p) -> p c", p=128))
    nc.sync.dma_start(bk_sb[:], bk.rearrange("(c p) -> p c", p=128))
    bv_bc = const.tile([128, DC], F32, tag="bv")
    nc.sync.dma_start(bv_bc[:], bv.unsqueeze(0).to_broadcast([128, DC]))
    ones4 = const.tile([128, HC], F32, tag="ones4")
    nc.vector.memset(ones4[:], 1.0)

    # Activations for phases 1-2 (freed before phase 3).
    acts_ctx = ExitStack()
    acts = acts_ctx.enter_context(tc.tile_pool(name="acts", bufs=1))
    QT = [acts.tile([128, S], F32R, tag=f"qt{p}", name=f"qt{p}") for p in range(NP)]
    KT_ = [acts.tile([128, S], F32R, tag=f"kt{p}", name=f"ktile{p}") for p in range(NP)]
    VPA = acts.tile([128, KT, HC * 65], F32R, tag="vpa", name="vpa")
    VP = [VPA[:, k, :] for k in range(KT)]
    # ctx'' spills to DRAM between phase 2 and phase 3.
    ctx_dram = nc.dram_tensor("ctxs", [HC, 65, S], F32).ap()

    # ---------------- Phase 1: X^T + projections ----------------
    with tc.tile_pool(name="xrow", bufs=6) as xrow, \
         tc.tile_pool(name="xt", bufs=16) as xtp, \
         tc.tile_pool(name="ps1", bufs=2, space="PSUM") as ps1, \
         tc.tile_pool(name="ps2", bufs=2, space="PSUM") as ps2:
        for t8 in range(TC):
            xts = [xrow.tile([128, D], F32, tag="xr", name=f"xr{t8}_{i}") for i in range(4)]
            for tt in range(4):
                nc.sync.dma_start(xts[tt][:], X[t8 * 512 + tt * 128:
                                              t8 * 512 + (tt + 1) * 128, :])
            xt = [xtp.tile([128, 512], F32R, tag="xt", name=f"xt{t8}_{i}") for i in range(8)]
            for dc in range(8):
                for tt in range(4):
                    pt = ps1.tile([128, 128], F32, tag="tp")
                    nc.tensor.transpose(
                        pt[:], xts[tt][:, dc * 128:(dc + 1) * 128],
                        ident[:])
                    nc.vector.tensor_copy(xt[dc][:, tt * 128:(tt + 1) * 128],
                                          pt[:])
            for p in range(NP):
                pq = ps2.tile([128, 512], F32, tag="pq")
                for dc in range(8):
                    nc.tensor.matmul(
                        pq[:], wq_sb[:, dc * DC + p * 128: dc * DC + (p + 1) * 128],
                        xt[dc][:], start=(dc == 0), stop=(dc == 7))
                nc.vector.tensor_scalar_add(
                    QT[p][:, t8 * 512:(t8 + 1) * 512], pq[:], bq_sb[:, p:p + 1])
                pk = ps2.tile([128, 512], F32, tag="pq")
                for dc in range(8):
                    nc.tensor.matmul(
                        pk[:], wk_sb[:, dc * DC + p * 128: dc * DC + (p + 1) * 128],
                        xt[dc][:], start=(dc == 0), stop=(dc == 7))
                nc.vector.tensor_scalar_add(
                    KT_[p][:, t8 * 512:(t8 + 1) * 512], pk[:], bk_sb[:, p:p + 1])
            for tt in range(4):
                kt = t8 * 4 + tt
                pv = ps2.tile([128, 256], F32, tag="pv")
                for dc in range(8):
                    nc.tensor.matmul(
                        pv[:], xt[dc][:, tt * 128:(tt + 1) * 128],
                        wv_sb[:, dc * DC:(dc + 1) * DC],
                        start=(dc == 0), stop=(dc == 7))
                vdst = VP[kt][:].rearrange("p (h w) -> p h w", h=HC)[:, :, 0:64]
                nc.vector.scalar_tensor_tensor(
                    vdst, pv[:].rearrange("p (h w) -> p h w", h=HC), 1.0,
                    bv_bc[:].rearrange("p (h w) -> p h w", h=HC),
                    mybir.AluOpType.bypass, mybir.AluOpType.add)
                ones = VP[kt][:].rearrange("p (h w) -> p h w", h=HC)[:, :, 64:65]
                nc.vector.tensor_copy(ones, ones4[:].unsqueeze(2))

    # ---------------- Phase 2: attention ----------------
    with tc.tile_pool(name="sps", bufs=2, space="PSUM") as sps, \
         tc.tile_pool(name="pvs", bufs=4, space="PSUM") as pvs, \
         tc.tile_pool(name="et", bufs=3) as etp, \
         tc.tile_pool(name="bsb", bufs=2) as bsb:
        for p in range(NP):
            for qc in range(QC):
                qs = slice(qc * 512, (qc + 1) * 512)
                acc = [pvs.tile([65, 512], F32, tag="acc", name=f"acc{p}_{qc}_{i}") for i in range(4)]
                for k in range(KT):
                    ks = slice(k * 128, (k + 1) * 128)
                    st = sps.tile([128, 1024], F32, tag="st")
                    nc.tensor.matmul(st[:, 0:512], KT_[p][0:64, ks],
                                     QT[p][0:64, qs], start=True, stop=True)
                    nc.tensor.matmul(st[:, 512:1024], KT_[p][64:128, ks],
                                     QT[p][64:128, qs], start=True, stop=True)
                    et = etp.tile([128, 1024], F32R, tag="et")
                    nc.scalar.activation(et[:], st[:], AF.Exp, bias=0.0,
                                         scale=SCALE)
                    for j in range(2):
                        h = 2 * p + j
                        vs = slice(h * 65, (h + 1) * 65)
                        es = slice(j * 512, (j + 1) * 512)
                        nc.tensor.matmul(
                            acc[2 * j][:], VP[k][0:64, vs], et[0:64, es],
                            start=(k == 0), stop=(k == KT - 1),
                            skip_group_check=True)
                        nc.tensor.matmul(
                            acc[2 * j + 1][:], VP[k][64:128, vs],
                            et[64:128, es],
                            start=(k == 0), stop=(k == KT - 1),
                            skip_group_check=True)
                for j in range(2):
                    h = 2 * p + j
                    btmp = bsb.tile([65, 512], F32, tag="btmp")
                    nc.vector.tensor_copy(btmp[:], acc[2 * j + 1][:])
                    cst = bsb.tile([65, 512], F32, tag="cst")
                    nc.vector.tensor_add(cst[:], acc[2 * j][:], btmp[:])
                    nc.sync.dma_start(ctx_dram[h, :, qs], cst[:])

    acts_ctx.close()

    # ---------------- Phase 3: normalize + Wo ----------------
    with tc.tile_pool(name="ps3", bufs=2, space="PSUM") as ps3, \
         tc.tile_pool(name="po", bufs=2, space="PSUM") as pop, \
         tc.tile_pool(name="sb3", bufs=4) as sb3, \
         tc.tile_pool(name="ctl", bufs=8) as ctl, \
         tc.tile_pool(name="osb", bufs=3) as osbp:
        for t in range(S // 128):
            ts_ = slice(t * 128, (t + 1) * 128)
            lts = []
            for p in range(NP):
                ctxn = sb3.tile([128, 128], F32, tag="ctxn")
                for j in range(2):
                    h = 2 * p + j
                    ct = ctl.tile([65, 128], F32, tag="ct")
                    nc.sync.dma_start(ct[:], ctx_dram[h, :, ts_])
                    tp1 = ps3.tile([128, 65], F32, tag="tp1")
                    nc.tensor.transpose(tp1[:], ct[:],
                                        ident[0:65, 0:65])
                    rcp = sb3.tile([128, 1], F32, tag="rcp")
                    nc.vector.reciprocal(rcp[:], tp1[:, 64:65])
                    nc.vector.tensor_scalar_mul(
                        ctxn[:, j * 64:(j + 1) * 64], tp1[:, 0:64], rcp[:])
                tp2 = ps3.tile([128, 128], F32, tag="tp2")
                nc.tensor.transpose(tp2[:], ctxn[:], ident[:])
                lt = sb3.tile([128, 128], F32R, tag="lt")
                nc.vector.tensor_copy(lt[:], tp2[:])
                lts.append(lt)
            for n2 in range(2):
                po = pop.tile([128, 512], F32, tag="po")
                for p in range(NP):
                    nc.tensor.matmul(
                        po[:], lts[p][:],
                        wo_sb[:, p * D + n2 * 512: p * D + (n2 + 1) * 512],
                        start=(p == 0), stop=(p == NP - 1))
                ot = osbp.tile([128, 512], F32, tag="ot")
                nc.vector.tensor_copy(ot[:], po[:])
                nc.sync.dma_start(out[ts_, n2 * 512:(n2 + 1) * 512], ot[:])


_CACHE = {}


def _build():
    if "nc" in _CACHE:
        return _CACHE["nc"]
    nc = bacc.Bacc("TRN2", target_bir_lowering=False, debug=False)
    ins = {
        "X": nc.dram_tensor("X", [S, D], F32, kind="ExternalInput").ap(),
        "Wq": nc.dram_tensor("Wq", [D, DC], F32, kind="ExternalInput").ap(),
        "bq": nc.dram_tensor("bq", [DC], F32, kind="ExternalInput").ap(),
        "Wk": nc.dram_tensor("Wk", [D, DC], F32, kind="ExternalInput").ap(),
        "bk": nc.dram_tensor("bk", [DC], F32, kind="ExternalInput").ap(),
        "Wv": nc.dram_tensor("Wv", [D, DC], F32, kind="ExternalInput").ap(),
        "bv": nc.dram_tensor("bv", [DC], F32, kind="ExternalInput").ap(),
        "Wo": nc.dram_tensor("Wo", [DC, D], F32, kind="ExternalInput").ap(),
    }
    outp = nc.dram_tensor("out", [S, D], F32, kind="ExternalOutput").ap()
    with tile.TileContext(nc) as tcx:
        with ExitStack() as ctx:
            _emit(ctx, tcx, ins, outp)
    nc.compile()
    _CACHE["nc"] = nc
    return nc


def core_inputs(X, Wq, bq, Wk, bk, Wv, bv, Wo, core):
    b, g = divmod(core, 4)
    cs = slice(g * DC, (g + 1) * DC)
    return {
        "X": np.ascontiguousarray(X[b]),
        "Wq": np.ascontiguousarray(Wq[:, cs]), "bq": np.ascontiguousarray(bq[cs]),
        "Wk": np.ascontiguousarray(Wk[:, cs]), "bk": np.ascontiguousarray(bk[cs]),
        "Wv": np.ascontiguousarray(Wv[:, cs]), "bv": np.ascontiguousarray(bv[cs]),
        "Wo": np.ascontiguousarray(Wo[cs, :]),
    }


def kernel(X, Wq, bq, Wk, bk, Wv, bv, Wo, bo, _trace=False):
    nc = _build()
    in_maps = [core_inputs(X, Wq, bq, Wk, bk, Wv, bv, Wo, c) for c in range(8)]
    res = run_bass_kernel_spmd(nc, in_maps, list(range(8)), trace=_trace)
    parts = [res.results[c]["out"] for c in range(8)]
    full = np.stack([
        parts[0] + parts[1] + parts[2] + parts[3] + bo,
        parts[4] + parts[5] + parts[6] + parts[7] + bo,
    ]).astype(np.float32)
    if _trace:
        return full, res
    return full


# revision 8
# speedup vs baseline: 1.2371x; 1.2371x over previous
"""Multi-head attention (B=2, S=4096, D=1024, H=16, HD=64) on 8 trn2 cores.

Sharding: core c -> batch b = c//4, head-group g = c%4 (4 heads per core).
Each core: Q/K/V projections for its heads on its batch, attention, and the
partial output ctx @ Wo[rows of its heads]. Host sums the 4 partials per
batch and adds bo.

Per-core kernel layout choices:
  - X^T [D, tok] built on-chip via PE transposes (DMA transpose is 2B-only).
  - Q^T, K^T stored [dq, tok] with each head-pair packed on partitions
    (head 2p   -> partitions  0:64,
     head 2p+1 -> partitions 64:128) so scores matmuls run as two
    concurrent 64-row PE tiles (full PE rate despite HD=64 contraction).
  - Scores computed transposed: S^T[k, q] = K^T.T @ Q^T. Softmax skips the
    max-subtraction (scores are O(5), exp cannot overflow) so exp is a
    single scalar-engine pass PSUM->SBUF with the 1/8 scale folded in.
  - V stored token-major with a ones column appended per head (V'[k, 65]).
    PV matmul (row-tiled over k halves) then yields ctx^T[d, q] and the
    softmax denominator row in one accumulation.
  - Normalization: transpose ctx''[65, q-tile] -> [q, 65], multiply by
    reciprocal of the sums column (per-partition scalar), transpose back,
    feed as lhsT of the Wo matmul.
All matmuls run as float32r (full-rate fp32 at free dim >= 256).
"""

import os
from contextlib import ExitStack

import numpy as np

os.environ.setdefault("MYCRO_LOCAL_CACHE", "1")

import concourse.bass as bass
import concourse.tile as tile
from concourse import bacc, mybir
from concourse.bass_utils import run_bass_kernel_spmd
from concourse.masks import make_identity

F32 = mybir.dt.float32
F32R = mybir.dt.float32r
AF = mybir.ActivationFunctionType

S = 4096          # sequence length
D = 1024          # model dim
HC = 4            # heads per core
HD = 64           # head dim
DC = HC * HD      # 256 per-core projection width
NP = HC // 2      # head pairs per core
KT = S // 128     # 32 k-tiles
QC = S // 512     # 8 q-chunks of 512
TC = S // 512     # 8 tok-chunks of 512
SCALE = 1.0 / 8.0


def r(ap):
    return ap.bitcast(F32R)


def _emit(ctx: ExitStack, tc: tile.TileContext, ins: dict, out: bass.AP):
    nc = tc.nc
    X, Wq, bq, Wk, bk, Wv, bv, Wo = (
        ins["X"], ins["Wq"], ins["bq"], ins["Wk"], ins["bk"], ins["Wv"],
        ins["bv"], ins["Wo"],
    )

    const = ctx.enter_context(tc.tile_pool(name="const", bufs=1))
    ident = const.tile([128, 128], F32)
    make_identity(nc, ident[:])

    # Weights DMA'd as fp32 then converted to f32r (fp32r matmul operands
    # must be produced pre-rounded by a compute engine; DMA cannot round).
    wq_sb = const.tile([128, 8 * DC], F32R, tag="wq")
    wk_sb = const.tile([128, 8 * DC], F32R, tag="wk")
    wv_sb = const.tile([128, 8 * DC], F32R, tag="wv")
    wo_sb = const.tile([128, 2 * D], F32R, tag="wo")
    with tc.tile_pool(name="wstage", bufs=2) as wst:
        for dst, src, nchunks in ((wq_sb, Wq, 8), (wk_sb, Wk, 8),
                                  (wv_sb, Wv, 8), (wo_sb, Wo, 2)):
            stg = wst.tile([128, dst.shape[1]], F32, tag="wstg",
                           name=f"wstg_{src.name}")
            nc.sync.dma_start(stg[:].rearrange("p (c d) -> p c d", c=nchunks),
                              src.rearrange("(c p) d -> p c d", p=128))
            nc.vector.tensor_copy(dst[:], stg[:])
    bq_sb = const.tile([128, 2], F32, tag="bq")
    bk_sb = const.tile([128, 2], F32, tag="bk")
    nc.sync.dma_start(bq_sb[:], bq.rearrange("(c p) -> p c", p=128))
    nc.sync.dma_start(bk_sb[:], bk.rearrange("(c p) -> p c", p=128))
    bv_bc = const.tile([128, DC], F32, tag="bv")
    nc.sync.dma_start(bv_bc[:], bv.unsqueeze(0).to_broadcast([128, DC]))
    ones4 = const.tile([128, HC], F32, tag="ones4")
    nc.vector.memset(ones4[:], 1.0)

    # Activations for phases 1-2 (freed before phase 3).
    acts_ctx = ExitStack()
    acts = acts_ctx.enter_context(tc.tile_pool(name="acts", bufs=1))
    QT = [acts.tile([128, S], F32R, tag=f"qt{p}", name=f"qt{p}") for p in range(NP)]
    KT_ = [acts.tile([128, S], F32R, tag=f"kt{p}", name=f"ktile{p}") for p in range(NP)]
    VPA = acts.tile([128, KT, HC * 65], F32R, tag="vpa", name="vpa")
    VP = [VPA[:, k, :] for k in range(KT)]
    # ctx'' spills to DRAM between phase 2 and phase 3.
    ctx_dram = nc.dram_tensor("ctxs", [HC, 65, S], F32).ap()

    # ---------------- Phase 1: X^T + projections ----------------
    with nc.named_scope("ph1"), \
         tc.tile_pool(name="xrow", bufs=6) as xrow, \
         tc.tile_pool(name="xt", bufs=16) as xtp, \
         tc.tile_pool(name="ps1", bufs=2, space="PSUM") as ps1, \
         tc.tile_pool(name="ps2", bufs=2, space="PSUM") as ps2:
        for t8 in range(TC):
            xts = [xrow.tile([128, D], F32, tag="xr", name=f"xr{t8}_{i}") for i in range(4)]
            for tt in range(4):
                nc.sync.dma_start(xts[tt][:], X[t8 * 512 + tt * 128:
                                              t8 * 512 + (tt + 1) * 128, :])
            xt = [xtp.tile([128, 512], F32R, tag="xt", name=f"xt{t8}_{i}") for i in range(8)]
            for dc in range(8):
                for tt in range(4):
                    pt = ps1.tile([128, 128], F32, tag="tp")
                    nc.tensor.transpose(
                        pt[:], xts[tt][:, dc * 128:(dc + 1) * 128],
                        ident[:])
                    nc.vector.tensor_copy(xt[dc][:, tt * 128:(tt + 1) * 128],
                                          pt[:])
            for p in range(NP):
                pq = ps2.tile([128, 512], F32, tag="pq")
                for dc in range(8):
                    nc.tensor.matmul(
                        pq[:], wq_sb[:, dc * DC + p * 128: dc * DC + (p + 1) * 128],
                        xt[dc][:], start=(dc == 0), stop=(dc == 7))
                nc.vector.tensor_scalar_add(
                    QT[p][:, t8 * 512:(t8 + 1) * 512], pq[:], bq_sb[:, p:p + 1])
                pk = ps2.tile([128, 512], F32, tag="pq")
                for dc in range(8):
                    nc.tensor.matmul(
                        pk[:], wk_sb[:, dc * DC + p * 128: dc * DC + (p + 1) * 128],
                        xt[dc][:], start=(dc == 0), stop=(dc == 7))
                nc.vector.tensor_scalar_add(
                    KT_[p][:, t8 * 512:(t8 + 1) * 512], pk[:], bk_sb[:, p:p + 1])
            for tt in range(4):
                kt = t8 * 4 + tt
                pv = ps2.tile([128, 256], F32, tag="pv")
                for dc in range(8):
                    nc.tensor.matmul(
                        pv[:], xt[dc][:, tt * 128:(tt + 1) * 128],
                        wv_sb[:, dc * DC:(dc + 1) * DC],
                        start=(dc == 0), stop=(dc == 7))
                vdst = VP[kt][:].rearrange("p (h w) -> p h w", h=HC)[:, :, 0:64]
                nc.vector.scalar_tensor_tensor(
                    vdst, pv[:].rearrange("p (h w) -> p h w", h=HC), 1.0,
                    bv_bc[:].rearrange("p (h w) -> p h w", h=HC),
                    mybir.AluOpType.bypass, mybir.AluOpType.add)
                ones = VP[kt][:].rearrange("p (h w) -> p h w", h=HC)[:, :, 64:65]
                nc.vector.tensor_copy(ones, ones4[:].unsqueeze(2))

    # ---------------- Phase 2: attention ----------------
    with nc.named_scope("ph2"), \
         tc.tile_pool(name="sps", bufs=2, space="PSUM") as sps, \
         tc.tile_pool(name="pvs", bufs=4, space="PSUM") as pvs, \
         tc.tile_pool(name="et", bufs=4) as etp, \
         tc.tile_pool(name="bsb", bufs=2) as bsb:
        for p in range(NP):
            for qc in range(QC):
                qs = slice(qc * 512, (qc + 1) * 512)
                acc = [pvs.tile([65, 512], F32, tag="acc", name=f"acc{p}_{qc}_{i}") for i in range(4)]

                def s_step(k):
                    ks = slice(k * 128, (k + 1) * 128)
                    st = sps.tile([128, 1024], F32, tag="st",
                                  name=f"st{p}_{qc}_{k}")
                    nc.tensor.matmul(st[:, 0:512], KT_[p][0:64, ks],
                                     QT[p][0:64, qs], start=True, stop=True)
                    nc.tensor.matmul(st[:, 512:1024], KT_[p][64:128, ks],
                                     QT[p][64:128, qs], start=True, stop=True)
                    et = etp.tile([128, 1024], F32R, tag="et",
                                  name=f"et{p}_{qc}_{k}")
                    nc.scalar.activation(et[:], st[:], AF.Exp, bias=0.0,
                                         scale=SCALE)
                    return et

                ets = {0: s_step(0)}
                for k in range(KT):
                    if k + 1 < KT:
                        ets[k + 1] = s_step(k + 1)
                    et = ets.pop(k)
                    for j in range(2):
                        h = 2 * p + j
                        vs = slice(h * 65, (h + 1) * 65)
                        es = slice(j * 512, (j + 1) * 512)
                        nc.tensor.matmul(
                            acc[2 * j][:], VP[k][0:64, vs], et[0:64, es],
                            start=(k == 0), stop=(k == KT - 1),
                            skip_group_check=True)
                        nc.tensor.matmul(
                            acc[2 * j + 1][:], VP[k][64:128, vs],
                            et[64:128, es],
                            start=(k == 0), stop=(k == KT - 1),
                            skip_group_check=True)
                for j in range(2):
                    h = 2 * p + j
                    btmp = bsb.tile([65, 512], F32, tag="btmp")
                    nc.vector.tensor_copy(btmp[:], acc[2 * j + 1][:])
                    cst = bsb.tile([65, 512], F32, tag="cst")
                    nc.vector.tensor_add(cst[:], acc[2 * j][:], btmp[:])
                    nc.sync.dma_start(ctx_dram[h, :, qs], cst[:])

    acts_ctx.close()

    # ---------------- Phase 3: normalize + Wo ----------------
    with nc.named_scope("ph3"), \
         tc.tile_pool(name="ps3", bufs=2, space="PSUM") as ps3, \
         tc.tile_pool(name="po", bufs=2, space="PSUM") as pop, \
         tc.tile_pool(name="sb3", bufs=4) as sb3, \
         tc.tile_pool(name="ctl", bufs=8) as ctl, \
         tc.tile_pool(name="osb", bufs=3) as osbp:
        for t in range(S // 128):
            ts_ = slice(t * 128, (t + 1) * 128)
            lts = []
            for p in range(NP):
                ctxn = sb3.tile([128, 128], F32, tag="ctxn")
                for j in range(2):
                    h = 2 * p + j
                    ct = ctl.tile([65, 128], F32, tag="ct")
                    nc.sync.dma_start(ct[:], ctx_dram[h, :, ts_])
                    tp1 = ps3.tile([128, 65], F32, tag="tp1")
                    nc.tensor.transpose(tp1[:], ct[:],
                                        ident[0:65, 0:65])
                    rcp = sb3.tile([128, 1], F32, tag="rcp")
                    nc.vector.reciprocal(rcp[:], tp1[:, 64:65])
                    nc.vector.tensor_scalar_mul(
                        ctxn[:, j * 64:(j + 1) * 64], tp1[:, 0:64], rcp[:])
                tp2 = ps3.tile([128, 128], F32, tag="tp2")
                nc.tensor.transpose(tp2[:], ctxn[:], ident[:])
                lt = sb3.tile([128, 128], F32R, tag="lt")
                nc.vector.tensor_copy(lt[:], tp2[:])
                lts.append(lt)
            for n2 in range(2):
                po = pop.tile([128, 512], F32, tag="po")
                for p in range(NP):
                    nc.tensor.matmul(
                        po[:], lts[p][:],
                        wo_sb[:, p * D + n2 * 512: p * D + (n2 + 1) * 512],
                        start=(p == 0), stop=(p == NP - 1))
                ot = osbp.tile([128, 512], F32, tag="ot")
                nc.vector.tensor_copy(ot[:], po[:])
                nc.sync.dma_start(out[ts_, n2 * 512:(n2 + 1) * 512], ot[:])


_CACHE = {}


def _build():
    if "nc" in _CACHE:
        return _CACHE["nc"]
    nc = bacc.Bacc("TRN2", target_bir_lowering=False, debug=False)
    ins = {
        "X": nc.dram_tensor("X", [S, D], F32, kind="ExternalInput").ap(),
        "Wq": nc.dram_tensor("Wq", [D, DC], F32, kind="ExternalInput").ap(),
        "bq": nc.dram_tensor("bq", [DC], F32, kind="ExternalInput").ap(),
        "Wk": nc.dram_tensor("Wk", [D, DC], F32, kind="ExternalInput").ap(),
        "bk": nc.dram_tensor("bk", [DC], F32, kind="ExternalInput").ap(),
        "Wv": nc.dram_tensor("Wv", [D, DC], F32, kind="ExternalInput").ap(),
        "bv": nc.dram_tensor("bv", [DC], F32, kind="ExternalInput").ap(),
        "Wo": nc.dram_tensor("Wo", [DC, D], F32, kind="ExternalInput").ap(),
    }
    outp = nc.dram_tensor("out", [S, D], F32, kind="ExternalOutput").ap()
    with tile.TileContext(nc) as tcx:
        with ExitStack() as ctx:
            _emit(ctx, tcx, ins, outp)
    nc.compile()
    _CACHE["nc"] = nc
    return nc


def core_inputs(X, Wq, bq, Wk, bk, Wv, bv, Wo, core):
    b, g = divmod(core, 4)
    cs = slice(g * DC, (g + 1) * DC)
    return {
        "X": np.ascontiguousarray(X[b]),
        "Wq": np.ascontiguousarray(Wq[:, cs]), "bq": np.ascontiguousarray(bq[cs]),
        "Wk": np.ascontiguousarray(Wk[:, cs]), "bk": np.ascontiguousarray(bk[cs]),
        "Wv": np.ascontiguousarray(Wv[:, cs]), "bv": np.ascontiguousarray(bv[cs]),
        "Wo": np.ascontiguousarray(Wo[cs, :]),
    }


def kernel(X, Wq, bq, Wk, bk, Wv, bv, Wo, bo, _trace=False):
    nc = _build()
    in_maps = [core_inputs(X, Wq, bq, Wk, bk, Wv, bv, Wo, c) for c in range(8)]
    res = run_bass_kernel_spmd(nc, in_maps, list(range(8)), trace=_trace)
    parts = [res.results[c]["out"] for c in range(8)]
    full = np.stack([
        parts[0] + parts[1] + parts[2] + parts[3] + bo,
        parts[4] + parts[5] + parts[6] + parts[7] + bo,
    ]).astype(np.float32)
    if _trace:
        return full, res
    return full
